# revision 27
# baseline (speedup 1.0000x reference)
"""Complex multihead attention Trainium2 kernel (fp8 DoubleRow, pipelined).

Math (per batch b):
  Q = qc @ Wq^T + bq ; K = kc @ Wk^T (+bk dropped: a per-query constant
      shift of scores cancels exactly in softmax) ; V = vc @ Wv^T + bv
  per head h (dh=64): S[q,k] = sum_d Qh[q,d] * Kh[k,d] / sqrt(dh)
  A = softmax(S.real, k) + i*softmax(S.imag, k)
  U = A @ Vh ; out = layernorm_r/i(U.merge_heads + qc)

Sharding: core c handles batch b=c//2, query half c%2 (512 queries).
K/V projections for the batch are computed on both cores of a pair
(duplicated) so there is no cross-core communication at all.

fp8 scheme: weights are scaled by 32 on host and cast to fp8e4 (e4m3).
x inputs are cast to fp8 (and pre-transposed) on host. All projection /
scores / AV matmuls run in fp8; projections and AV use DoubleRow perf
mode (contraction 256 per pass). PSUM accumulation stays fp32. The
32x scale cancels: scores exp folds 1/(32*32*sqrt(dh)) into the ACT
scale; V's 32x is divided out with the softmax denominator (rinv is
1/(32*r)).

Pipeline: per-head emission interleaves head h's scores/exp with head
h-1's AV/r matmuls so the PE never stalls on the ACT exp; V projection
is interleaved into head 0's scores. PSUM: sc(2x2) + pj(2)/raux(2) +
p1 + p2 = 8 banks (pj pool closes before raux opens).

On-chip layouts (per core):
  xqT [128, 8, 512], xkT/xvT [128, 8, 1024] fp8: dim1 = chunk
      (ci<4 -> Xr^T d-block ci, ci>=4 -> Xi^T d-block ci-4), host-prepped.
  Weights fp8 (host-prepped, dim1 = d-in sub-block ds):
    wq_a/wk_a [128, 4, 2048]: pattern1 (Wr^T|Wi^T) head-paired cols
        0:1024, pattern2 (-Wi^T|Wr^T) cols 1024:2048 (x32, fp8)
    wq_b [128, 4, 2048]: pattern3 (Wi^T|Wr^T), pattern4 (Wr^T|-Wi^T)
    wv_s [128, 4, 1536]: plain Wr^T | Wi^T | -Wi^T  (512-col slots)
  DoubleRow: lhsT/rhs sliced [:, ch:ch+2, :] pair two 128-row
      contraction chunks; one matmul contracts 256 rows.
  K_stk [128, 8h*1024] fp8: rows 0:64 Kr^T(head cols), 64:128 Ki^T
  Qv1   [128, 8h*512] fp8:  [Qr^T; -Qi^T] per head  (bias folded)
  Qv2   [128, 8h*512] fp8:  [Qi^T;  Qr^T] per head  (bias folded)
  V_all [128, 8kt, 1024] fp8: per k-tile head-paired cols [Vr_h 64|Vi_h 64]
  scores^T psum [128 k, 1024] fp32: cols 0:512 Sr^T, 512:1024 Si^T per kt
  E pair tiles [128, 2, 1024] fp8 = exp(S^T/(1024*8)), dim1 = kt in pair
  P1 psum [128,512] = sum_ktp [Vr|Vi]^T Er (DoubleRow) -> [ErVr; ErVi]
  P2 psum [128,512] -> [EiVr; EiVi]
  r psum [1, 1024]: cols 0:512 sum_k Er, 512: sum_k Ei (ones DR matmuls),
      fanned out to rsb [2, 512] by DMA straight from PSUM.
  OUT_int [128, 4qs, 1024] fp32: final (d,c)-interleaved output tile,
      pre-initialized with residual qc + (1+i)*bv; per-head evac
      accumulates into it (scalar_tensor_tensor adds).
"""

import math
from contextlib import ExitStack

import numpy as np
import ml_dtypes

import concourse.bass as bass
import concourse.tile as tile
from concourse import bacc, mybir
from concourse.bass_utils import run_bass_kernel_spmd
from concourse.masks import make_identity

B, S, D, H = 4, 1024, 512, 8
DH = D // H  # 64
NCORES = 8
TQ = S // 2  # queries per core
EPS = 1e-5
F32 = mybir.dt.float32
F32R = mybir.dt.float32r
BF16 = mybir.dt.bfloat16
F8 = mybir.dt.float8e4
NPF8 = ml_dtypes.float8_e4m3

WS = 32.0  # host-side weight scale (power of 2)
F8MAX = 240.0  # e4m3 (IEEE) max finite

NKT = S // 128  # 8 key tiles
NQS = TQ // 128  # 4 query subtiles
NCH = 8  # contraction chunks (2*D/128)
DR = mybir.MatmulPerfMode.DoubleRow


def build_nc(skip_gb: bool) -> bass.Bass:
    nc = bacc.Bacc(None, target_bir_lowering=False, debug=False)

    xqT_d = nc.declare_dram_parameter("xqT", [128, NCH, TQ], F8, isOutput=False)
    xkT_d = nc.declare_dram_parameter("xkT", [128, NCH, S], F8, isOutput=False)
    xvT_d = nc.declare_dram_parameter("xvT", [128, NCH, S], F8, isOutput=False)
    xqn_d = nc.declare_dram_parameter("xq_nat", [TQ, 2 * D], F32, isOutput=False)
    wqa_d = nc.declare_dram_parameter("wq_a", [128, 8, 1024], F8, isOutput=False)
    wka_d = nc.declare_dram_parameter("wk_a", [128, 4, 2048], F8, isOutput=False)
    wvs_d = nc.declare_dram_parameter("wv_s", [128, 4, 1536], F8, isOutput=False)
    bq1_d = nc.declare_dram_parameter("bq1_stk", [128, H], F32, isOutput=False)
    bv_d = nc.declare_dram_parameter("bv_int", [1, 2 * D], F32, isOutput=False)
    gam_d = nc.declare_dram_parameter("gam_int", [1, 2 * D], F32, isOutput=False)
    bet_d = nc.declare_dram_parameter("bet_int", [1, 2 * D], F32, isOutput=False)
    out_d = nc.declare_dram_parameter("out", [TQ, D, 2], BF16, isOutput=True)

    with tile.TileContext(nc) as tc, ExitStack() as ctx:
        consts = ctx.enter_context(tc.tile_pool(name="consts", bufs=1))
        attn_in = ctx.enter_context(tc.tile_pool(name="attn_in", bufs=1))

        # tiny bias DMAs first (Q evac needs them early; a 2MB transfer
        # queued ahead of them stalls the whole Q stage)
        bq1_stk = consts.tile([128, H], F32)
        nc.sync.dma_start(out=bq1_stk, in_=bq1_d[:])

        # inputs: DMA in compute order (Q stage first so PE starts early);
        # wq_a is head-major so head 0's weights (128KB) land quickly
        wqa = attn_in.tile([128, 8, 1024], F8)
        nc.sync.dma_start(out=wqa[:, 0:1, :], in_=wqa_d[:, 0:1, :])
        xqT = attn_in.tile([128, NCH, TQ], F8)
        nc.sync.dma_start(out=xqT, in_=xqT_d[:])
        for _h in range(1, H):
            nc.sync.dma_start(out=wqa[:, _h : _h + 1, :], in_=wqa_d[:, _h : _h + 1, :])
        wka = attn_in.tile([128, 4, 2048], F8)
        nc.scalar.dma_start(out=wka, in_=wka_d[:])
        xkT = attn_in.tile([128, NCH, S], F8)
        nc.scalar.dma_start(out=xkT, in_=xkT_d[:])
        wvs = attn_in.tile([128, 4, 1536], F8)
        nc.sync.dma_start(out=wvs, in_=wvs_d[:])
        xvT = attn_in.tile([128, NCH, S], F8)
        nc.sync.dma_start(out=xvT, in_=xvT_d[:])

        ident_f = consts.tile([128, 128], F32)
        make_identity(nc, ident_f)
        ident = consts.tile([128, 128], BF16)
        nc.vector.tensor_copy(out=ident, in_=ident_f)
        ones_f = consts.tile([128, 32], F32)
        nc.vector.memset(ones_f, 1.0)
        ones8 = consts.tile([128, 32], F8)
        nc.vector.tensor_copy(out=ones8, in_=ones_f)
        # DoubleRow ones lhsT [128, 2, 1] (dim1 step 16B for alignment)
        ones_pair = bass.AP(
            tensor=ones8.tensor, offset=ones8.offset,
            ap=[ones8.ap[0], [16, 2], [1, 1]],
        )
        # [0,1] stationary: DR lhsT [128, 2, 2] whose output partition 0
        # receives +0 and partition 1 the ones-reduction (lets r_i share
        # r_r's psum bank one partition over)
        zer_f = consts.tile([128, 32], F32)
        nc.vector.memset(zer_f, 0.0)
        onesB = consts.tile([128, 32], F8)
        nc.vector.tensor_copy(out=onesB, in_=zer_f)
        oneB_col = bass.AP(
            tensor=onesB.tensor, offset=onesB.offset + 1,
            ap=[onesB.ap[0], [16, 2], [1, 1]],
        )
        nc.vector.tensor_copy(out=oneB_col, in_=ones_pair)
        onesB_pair = bass.AP(
            tensor=onesB.tensor, offset=onesB.offset,
            ap=[onesB.ap[0], [16, 2], [1, 2]],
        )
        eps_t = consts.tile([128, 1], F32)
        nc.vector.memset(eps_t, EPS)

        bv_bc = consts.tile([128, 2 * D], F32)
        bcs = [(bv_d, bv_bc)]
        if not skip_gb:
            gam_bc = consts.tile([128, 2 * D], F32)
            bet_bc = consts.tile([128, 2 * D], F32)
            bcs += [(gam_d, gam_bc), (bet_d, bet_bc)]
        for dram, bc in bcs:
            ap0 = dram[:]
            src = bass.AP(tensor=ap0.tensor, offset=0, ap=[[0, 128], [1, 2 * D]])
            nc.gpsimd.dma_start(out=bc, in_=src)

        # attention-phase operand tensors (filled by projection stages)
        K_stk = attn_in.tile([128, H * S], F8)
        V_all = attn_in.tile([128, NKT, 2 * D], F8)
        Qv1 = attn_in.tile([128, H * TQ], F8)
        Qv2 = attn_in.tile([128, H * TQ], F8)
        OUT_int = attn_in.tile([128, NQS, 2 * D], F32)

        # OUT_int <- residual qc (+ deferred (1+i)*bv, added on Pool)
        for qs in range(NQS):
            nc.sync.dma_start(
                out=OUT_int[:, qs, :], in_=xqn_d[qs * 128 : (qs + 1) * 128]
            )
        for qs in range(NQS):
            seg = OUT_int[:, qs, :]
            nc.gpsimd.tensor_add(out=seg, in0=seg, in1=bv_bc)

        def w_pair(w, h, p):
            """DoubleRow lhsT [128, 2, 128] for chunk pair (2p, 2p+1)."""
            pat = 0 if p < 2 else 1
            ds_ = (2 * p) % 4
            return w[:, ds_ : ds_ + 2, pat * 1024 + 128 * h : pat * 1024 + 128 * h + 128]

        def v_rhs(w, p, comp):
            """DoubleRow rhs [128, 2, 512] slot pair for V projection."""
            lo = p < 2
            s = {(0, True): 0, (0, False): 2, (1, True): 1, (1, False): 0}[(comp, lo)]
            ds_ = (2 * p) % 4
            return w[:, ds_ : ds_ + 2, s * 512 : (s + 1) * 512]

        e_pool = ctx.enter_context(tc.tile_pool(name="epool", bufs=7))
        r_pool = ctx.enter_context(tc.tile_pool(name="rsb", bufs=2))
        u_pool = ctx.enter_context(tc.tile_pool(name="usk", bufs=2))
        sc_psum = ctx.enter_context(tc.tile_pool(name="sc_ps", bufs=2, space="PSUM"))
        p1_psum = ctx.enter_context(tc.tile_pool(name="p1_ps", bufs=1, space="PSUM"))
        p2_psum = ctx.enter_context(tc.tile_pool(name="p2_ps", bufs=1, space="PSUM"))
        pj_cm = tc.tile_pool(name="pj_ps", bufs=2, space="PSUM")
        pj_psum = pj_cm.__enter__()

        qt_pool = ctx.enter_context(tc.tile_pool(name="qtmp", bufs=2))

        def q_proj():
            # single projection to [Qr+bqr; Qi+bqi]; Qv1 negates the bottom,
            # Qv2 = swapped halves, moved by two Pool-queue SBUF DMAs
            for h in range(H):
                ps = pj_psum.tile([128, TQ], F32, tag="pj")
                for p in range(4):
                    pat = 0 if p < 2 else 1
                    ds0 = (2 * p) % 4
                    lhsT = bass.AP(
                        tensor=wqa.tensor,
                        offset=wqa.offset + h * 1024 + ds0 * 256 + pat * 128,
                        ap=[wqa.ap[0], [256, 2], [1, 128]],
                    )
                    nc.tensor.matmul(
                        ps, lhsT, xqT[:, 2 * p : 2 * p + 2, :],
                        start=(p == 0), stop=(p == 3),
                        perf_mode=DR,
                    )
                sl1 = Qv1[:, h * TQ : (h + 1) * TQ]
                sl2 = Qv2[:, h * TQ : (h + 1) * TQ]
                qtmp = qt_pool.tile([128, TQ], F8, tag="qtmp", name="qtmp")
                nc.scalar.activation(
                    out=sl1[0:DH, :], in_=ps[0:DH, :],
                    func=mybir.ActivationFunctionType.Identity,
                    bias=bq1_stk[0:DH, h : h + 1],
                )
                nc.scalar.activation(
                    out=qtmp[DH:128, :], in_=ps[DH:128, :],
                    func=mybir.ActivationFunctionType.Identity,
                    bias=bq1_stk[DH:128, h : h + 1],
                )
                nc.vector.tensor_scalar_mul(
                    out=sl1[DH:128, :], in0=qtmp[DH:128, :], scalar1=-1.0
                )
                nc.gpsimd.dma_start(out=sl2[0:DH, :], in_=qtmp[DH:128, :])
                nc.gpsimd.dma_start(out=sl2[DH:128, :], in_=sl1[0:DH, :])

        def k_proj():
            for h in range(H):
                for tch in range(2):
                    ps = pj_psum.tile([128, 512], F32, tag="pj")
                    for p in range(4):
                        nc.tensor.matmul(
                            ps,
                            w_pair(wka, h, p),
                            xkT[:, 2 * p : 2 * p + 2, tch * 512 : (tch + 1) * 512],
                            start=(p == 0), stop=(p == 3),
                            perf_mode=DR,
                        )
                    nc.scalar.copy(
                        out=K_stk[:, h * S + tch * 512 : h * S + (tch + 1) * 512],
                        in_=ps,
                    )

        def v_proj(ts_lo, ts_hi):
            for ts_ in range(ts_lo, ts_hi):
                for comp in range(2):
                    ps = pj_psum.tile([128, 512], F32, tag="pj")
                    for p in range(4):
                        nc.tensor.matmul(
                            ps,
                            xvT[:, 2 * p : 2 * p + 2, ts_ * 128 : ts_ * 128 + 128],
                            v_rhs(wvs, p, comp),
                            start=(p == 0), stop=(p == 3),
                            perf_mode=DR,
                        )
                    # scatter into head-paired layout [Vr_h | Vi_h]
                    dst = bass.AP(
                        tensor=V_all.tensor,
                        offset=V_all.offset + ts_ * 2 * D + comp * DH,
                        ap=[V_all.ap[0], [2 * DH, H], [1, DH]],
                    )
                    nc.vector.tensor_copy(
                        out=dst, in_=ps.rearrange("p (h j) -> p h j", h=H)
                    )

        # ---------------- phase B: attention --------------------------------
        EXPSC = 1.0 / (WS * WS * math.sqrt(DH))
        etiles = {}
        pstate = {}

        def sc_ktp(h, ktp):
            """scores + exp for head h, kt pair ktp."""
            ep = e_pool.tile([128, 2, 1024], F8, tag="e")
            etiles[(h, ktp)] = ep
            for sub in range(2):
                kt = 2 * ktp + sub
                scp = sc_psum.tile([128, 1024], F32, tag="sc")
                klhs = K_stk[:, h * S + kt * 128 : h * S + kt * 128 + 128]
                nc.tensor.matmul(
                    scp[:, 0:TQ], klhs, Qv1[:, h * TQ : (h + 1) * TQ],
                    start=True, stop=True,
                )
                nc.tensor.matmul(
                    scp[:, TQ : 2 * TQ], klhs, Qv2[:, h * TQ : (h + 1) * TQ],
                    start=True, stop=True,
                )
                nc.scalar.activation(
                    out=ep[:, sub, :], in_=scp,
                    func=mybir.ActivationFunctionType.Exp,
                    scale=EXPSC,
                )

        def av_ktp(h, ktp, r_psum):
            """AV + r DoubleRow matmuls for head h, kt pair ktp."""
            if ktp == 0:
                pstate[h] = (
                    p1_psum.tile([128, TQ], F32, tag="p1", name="p1"),
                    p2_psum.tile([128, TQ], F32, tag="p2", name="p2"),
                    r_psum.tile([2, TQ], F32, tag="raux", name="rp"),
                )
            p1, p2, rp = pstate[h]
            ep = etiles.pop((h, ktp))
            vl = V_all[:, 2 * ktp : 2 * ktp + 2, 128 * h : 128 * h + 128]
            erp = ep[:, :, 0:TQ]
            eip = ep[:, :, TQ : 2 * TQ]
            st, sp = (ktp == 0), (ktp == 3)
            nc.tensor.matmul(p1, vl, erp, start=st, stop=sp, perf_mode=DR)
            nc.tensor.matmul(p2, vl, eip, start=st, stop=sp, perf_mode=DR)
            # r_i -> partition 1 (zero col pads partition 0 with +0; its
            # start=True zero runs BEFORE r_r's write), r_r -> partition 0;
            # both share one psum bank
            nc.tensor.matmul(
                rp[0:2, 0:TQ], onesB_pair, eip, start=st, stop=sp,
                perf_mode=DR, skip_group_check=True,
            )
            nc.tensor.matmul(
                rp[0:1, 0:TQ], ones_pair, erp, start=st, stop=sp,
                perf_mode=DR, skip_group_check=True,
            )

        def evac_head(h, r_psum, u_pool, last=False):
            """P1/P2 + r -> OUT_int accumulation for head h."""
            p1, p2, rp = pstate.pop(h)
            # P1 rows [ErVr; ErVi], P2 rows [EiVr; EiVi] (V carries WS)
            usk1 = u_pool.tile([128, TQ], BF16, tag="usk1")
            usk2 = u_pool.tile([128, TQ], BF16, tag="usk2")
            if last:
                # ACT and the sc psum banks are idle once the exp stream
                # drains; keep the tail chain off the busy DVE queue
                nc.scalar.copy(out=usk1, in_=p1)
                nc.scalar.copy(out=usk2, in_=p2)
            else:
                nc.vector.tensor_copy(out=usk1, in_=p1)
                nc.vector.tensor_copy(out=usk2, in_=p2)
            # r rows (x WS so rinv = 1/(WS*r)) -> SBUF, partition-preserving
            rtp = r_pool.tile([2, TQ], BF16, tag="rtp")
            if last:
                nc.scalar.mul(out=rtp, in_=rp, mul=WS)
            else:
                nc.vector.tensor_scalar_mul(out=rtp, in0=rp, scalar1=WS)
            # r transposes first (low cols of utp), then P1/P2 transposes
            utp = r_psum.tile([128, TQ], BF16, tag="utp", name="utp")
            # rinv per qs: [1/(WS r_r), 1/(WS r_i), -1/(WS r_i)]
            rinv = r_pool.tile([128, 3 * NQS], F32, tag="rinv")
            for qs in range(NQS):
                nc.tensor.transpose(
                    utp[:, qs * 2 : qs * 2 + 2],
                    rtp[:, qs * 128 : (qs + 1) * 128],
                    ident[0:2, 0:2],
                )
                nc.vector.reciprocal(
                    out=rinv[:, 3 * qs : 3 * qs + 2],
                    in_=utp[:, qs * 2 : qs * 2 + 2],
                )
                nc.vector.tensor_scalar_mul(
                    out=rinv[:, 3 * qs + 2 : 3 * qs + 3],
                    in0=rinv[:, 3 * qs + 1 : 3 * qs + 2],
                    scalar1=-1.0,
                )
            for qs in range(NQS):
                nc.tensor.transpose(
                    utp[:, qs * 128 : (qs + 1) * 128],
                    usk1[:, qs * 128 : (qs + 1) * 128],
                    ident,
                )
            for qs in range(NQS):
                # pass 1: OUT cols of head h (both comps) += P1t * (1/r_r)
                dst = bass.AP(
                    tensor=OUT_int.tensor,
                    offset=OUT_int.offset + qs * 2 * D + 2 * DH * h,
                    ap=[OUT_int.ap[0], [1, 2], [2, DH]],
                )
                nc.vector.scalar_tensor_tensor(
                    out=dst,
                    in0=utp[:, qs * 128 : (qs + 1) * 128]
                    .rearrange("p (b c) -> p b c", b=2),
                    scalar=rinv[:, 3 * qs : 3 * qs + 1],
                    in1=dst,
                    op0=mybir.AluOpType.mult,
                    op1=mybir.AluOpType.add,
                )
            utp2 = (
                sc_psum.tile([128, TQ], BF16, tag="sc", name="utp2")
                if last else utp
            )
            for qs in range(NQS):
                nc.tensor.transpose(
                    utp2[:, qs * 128 : (qs + 1) * 128],
                    usk2[:, qs * 128 : (qs + 1) * 128],
                    ident,
                )
            for qs in range(NQS):
                # pass 2: r-cols += (EiVi) * (-1/r_i) ; i-cols += (EiVr)/r_i
                for c, sidx in ((0, 2), (1, 1)):
                    dst = bass.AP(
                        tensor=OUT_int.tensor,
                        offset=OUT_int.offset + qs * 2 * D + 2 * DH * h + c,
                        ap=[OUT_int.ap[0], [2, DH]],
                    )
                    src_col = (1 - c) * DH  # r <- EiVi block, i <- EiVr
                    nc.vector.scalar_tensor_tensor(
                        out=dst,
                        in0=utp2[:, qs * 128 + src_col : qs * 128 + src_col + DH],
                        scalar=rinv[:, 3 * qs + sidx : 3 * qs + sidx + 1],
                        in1=dst,
                        op0=mybir.AluOpType.mult,
                        op1=mybir.AluOpType.add,
                    )
                if last:
                    ln_qs(qs)

        ln_pool = ctx.enter_context(tc.tile_pool(name="ln", bufs=8))

        def ln_qs(qs):
            seg = OUT_int[:, qs, :]
            stage = ln_pool.tile([128, 2 * D], BF16, tag="stage", name="stage")
            for c in range(2):
                x = bass.AP(
                    tensor=OUT_int.tensor,
                    offset=OUT_int.offset + qs * 2 * D + c,
                    ap=[OUT_int.ap[0], [2, D]],
                )
                xs = bass.AP(
                    tensor=stage.tensor,
                    offset=stage.offset + c,
                    ap=[stage.ap[0], [2, D]],
                )
                st = ln_pool.tile([128, 6], F32, tag="st", name="st")
                nc.vector.bn_stats(out=st, in_=x)
                mv = ln_pool.tile([128, 2], F32, tag="mv", name="mv")
                nc.vector.bn_aggr(out=mv, in_=st)
                rs = ln_pool.tile([128, 1], F32, tag="rs", name="rs")
                nc.scalar.activation(
                    out=rs, in_=mv[:, 1:2],
                    func=mybir.ActivationFunctionType.Sqrt,
                    bias=eps_t,
                )
                nc.vector.reciprocal(out=rs, in_=rs)
                nc.vector.tensor_scalar(
                    out=x if not skip_gb else xs, in0=x,
                    scalar1=mv[:, 0:1], scalar2=rs,
                    op0=mybir.AluOpType.subtract,
                    op1=mybir.AluOpType.mult,
                )
            if not skip_gb:
                nc.vector.tensor_mul(out=seg, in0=seg, in1=gam_bc)
                nc.vector.tensor_add(out=stage, in0=seg, in1=bet_bc)
            nc.sync.dma_start(
                out=out_d[qs * 128 : (qs + 1) * 128],
                in_=stage.rearrange("p (d c) -> p d c", c=2),
            )

        # ---------------- emission -----------------------------------------
        q_proj()
        k_proj()
        # head 0 scores interleaved with V projection (keeps PE busy while
        # ACT exps head 0)
        for ktp in range(4):
            sc_ktp(0, ktp)
            v_proj(ktp * 2, ktp * 2 + 2)
        # pj pool's PSUM banks are recycled for the r/utp pool
        pj_cm.__exit__(None, None, None)
        r_psum = ctx.enter_context(tc.tile_pool(name="r_ps", bufs=1, space="PSUM"))

        # steady state: head h scores overlap head h-1 AV on the PE stream
        for h in range(1, H):
            for ktp in range(4):
                sc_ktp(h, ktp)
                av_ktp(h - 1, ktp, r_psum)
            evac_head(h - 1, r_psum, u_pool)
        for ktp in range(4):
            av_ktp(H - 1, ktp, r_psum)
        evac_head(H - 1, r_psum, u_pool, last=True)
    nc.compile()
    return nc


def _f8(a):
    return np.clip(a, -F8MAX, F8MAX).astype(NPF8)


def _prep_w(W: np.ndarray):
    """W [D, D, 2] -> (w_a, w_b [128,4,2048], w_s [128,4,1536]) fp8, x WS.

    w_a: pattern1 (Wr^T|Wi^T) head-paired, pattern2 (-Wi^T|Wr^T)
    w_b: pattern3 (Wi^T|Wr^T), pattern4 (Wr^T|-Wi^T)
    w_s: Wr^T | Wi^T | -Wi^T (512-col slots)
    Row dim (d_in) split into 4 sub-blocks -> dim1.
    """
    wr = np.ascontiguousarray(W[:, :, 0].T) * WS  # [d_in, e]
    wi = np.ascontiguousarray(W[:, :, 1].T) * WS

    def paired(a, b):
        out = np.empty((D, 1024), np.float32)
        for h in range(H):
            out[:, 128 * h : 128 * h + DH] = a[:, DH * h : DH * (h + 1)]
            out[:, 128 * h + DH : 128 * h + 128] = b[:, DH * h : DH * (h + 1)]
        return out

    def blk(x):  # [512, n] -> [128, 4, n]
        n = x.shape[1]
        return np.ascontiguousarray(x.reshape(4, 128, n).transpose(1, 0, 2))

    w_a = np.concatenate([paired(wr, wi), paired(-wi, wr)], axis=1)
    w_b = np.concatenate([paired(wi, wr), paired(wr, -wi)], axis=1)
    w_s = np.concatenate([wr, wi, -wi], axis=1)
    return _f8(blk(w_a)), _f8(blk(w_b)), _f8(blk(w_s))


def _xt(x: np.ndarray):
    """x [T, D, 2] f32 -> stacked transposed fp8 [128, 8, T]."""
    T = x.shape[0]
    x8 = _f8(x)
    return np.ascontiguousarray(
        x8.reshape(T, 4, 128, 2).transpose(2, 3, 1, 0).reshape(128, NCH, T)
    )


def _stk(vr, vi):
    """two [H*DH] vectors -> [128, H]: rows 0:64 vr per head, 64:128 vi."""
    out = np.empty((128, H), np.float32)
    for h in range(H):
        out[0:DH, h] = vr[h * DH : (h + 1) * DH]
        out[DH:128, h] = vi[h * DH : (h + 1) * DH]
    return out


def _inter(a, b):
    out = np.empty((1, 2 * D), np.float32)
    out[0, 0::2] = a
    out[0, 1::2] = b
    return out


def host_inputs(inputs: dict) -> tuple[list[dict], bool]:
    q = np.ascontiguousarray(np.asarray(inputs["q"], np.float32))
    k = np.ascontiguousarray(np.asarray(inputs["k"], np.float32))
    v = np.ascontiguousarray(np.asarray(inputs["v"], np.float32))
    Wq, bq = np.asarray(inputs["Wq"], np.float32), np.asarray(inputs["bq"], np.float32)
    Wk = np.asarray(inputs["Wk"], np.float32)
    Wv, bv = np.asarray(inputs["Wv"], np.float32), np.asarray(inputs["bv"], np.float32)
    gr = np.asarray(inputs["gamma_r"], np.float32)
    gi = np.asarray(inputs["gamma_i"], np.float32)
    br = np.asarray(inputs["beta_r"], np.float32)
    bi = np.asarray(inputs["beta_i"], np.float32)
    skip_gb = bool(
        np.all(gr == 1.0) and np.all(gi == 1.0)
        and np.all(br == 0.0) and np.all(bi == 0.0)
    )

    wq_a, _, _ = _prep_w(Wq)
    # head-major repack: [128, 4ds, 2048(pat,128h,c)] -> [128, 8h, 1024(ds,pat,c)]
    wq_a = np.ascontiguousarray(
        wq_a.reshape(128, 4, 2, 8, 128)
        .transpose(0, 3, 1, 2, 4)
        .reshape(128, 8, 1024)
    )
    wk_a, _, _ = _prep_w(Wk)
    _, _, wv_s = _prep_w(Wv)

    shared = {
        "wq_a": wq_a, "wk_a": wk_a, "wv_s": wv_s,
        "bq1_stk": _stk(bq[:, 0], bq[:, 1]) * WS,
        # deferred V-bias: attn rows sum to (1+i), so bv enters as (1+i)*bv
        "bv_int": _inter(bv[:, 0] - bv[:, 1], bv[:, 0] + bv[:, 1]),
        "gam_int": _inter(gr, gi),
        "bet_int": _inter(br, bi),
    }
    in_maps = []
    for c in range(NCORES):
        b_, qh = c // 2, c % 2
        xq = q[b_, qh * TQ : (qh + 1) * TQ]
        in_maps.append(
            {
                "xqT": _xt(xq),
                "xkT": _xt(k[b_]),
                "xvT": _xt(v[b_]),
                "xq_nat": np.ascontiguousarray(xq.reshape(TQ, 2 * D)),
                **shared,
            }
        )
    return in_maps, skip_gb


_NC_CACHE = {}
LAST_RESULT = [None]  # BassKernelResults of the most recent kernel() call


def kernel(**inputs) -> np.ndarray:
    in_maps, skip_gb = host_inputs(inputs)
    if skip_gb not in _NC_CACHE:
        _NC_CACHE[skip_gb] = build_nc(skip_gb)
    nc = _NC_CACHE[skip_gb]
    _NC_CACHE["nc"] = nc  # for test.py tracing
    res = run_bass_kernel_spmd(nc, in_maps, list(range(NCORES)))
    LAST_RESULT[0] = res
    out = np.empty((B, S, D, 2), np.float32)
    for c in range(NCORES):
        b_, qh = c // 2, c % 2
        out[b_, qh * TQ : (qh + 1) * TQ] = np.asarray(
            res.results[c]["out"], dtype=np.float32
        )
    return out


# revision 28
# speedup vs baseline: 1.0471x; 1.0471x over previous
"""Complex multihead attention Trainium2 kernel (fp8 DoubleRow, pipelined).

Math (per batch b):
  Q = qc @ Wq^T + bq ; K = kc @ Wk^T (+bk dropped: a per-query constant
      shift of scores cancels exactly in softmax) ; V = vc @ Wv^T + bv
  per head h (dh=64): S[q,k] = sum_d Qh[q,d] * Kh[k,d] / sqrt(dh)
  A = softmax(S.real, k) + i*softmax(S.imag, k)
  U = A @ Vh ; out = layernorm_r/i(U.merge_heads + qc)

Sharding: core c handles batch b=c//2, query half c%2 (512 queries).
K/V projections for the batch are computed on both cores of a pair
(duplicated) so there is no cross-core communication at all.

fp8 scheme: weights are scaled by 32 on host and cast to fp8e4 (e4m3).
x inputs are cast to fp8 (and pre-transposed) on host. All projection /
scores / AV matmuls run in fp8; projections and AV use DoubleRow perf
mode (contraction 256 per pass). PSUM accumulation stays fp32. The
32x scale cancels: scores exp folds 1/(32*32*sqrt(dh)) into the ACT
scale; V's 32x is divided out with the softmax denominator (rinv is
1/(32*r)).

Pipeline: per-head emission interleaves head h's scores/exp with head
h-1's AV/r matmuls so the PE never stalls on the ACT exp; V projection
is interleaved into head 0's scores. PSUM: sc(2x2) + pj(2)/raux(2) +
p1 + p2 = 8 banks (pj pool closes before raux opens).

On-chip layouts (per core):
  xqT [128, 8, 512], xkT/xvT [128, 8, 1024] fp8: dim1 = chunk
      (ci<4 -> Xr^T d-block ci, ci>=4 -> Xi^T d-block ci-4), host-prepped.
  Weights fp8 (host-prepped, dim1 = d-in sub-block ds):
    wq_a/wk_a [128, 4, 2048]: pattern1 (Wr^T|Wi^T) head-paired cols
        0:1024, pattern2 (-Wi^T|Wr^T) cols 1024:2048 (x32, fp8)
    wq_b [128, 4, 2048]: pattern3 (Wi^T|Wr^T), pattern4 (Wr^T|-Wi^T)
    wv_s [128, 4, 1536]: plain Wr^T | Wi^T | -Wi^T  (512-col slots)
  DoubleRow: lhsT/rhs sliced [:, ch:ch+2, :] pair two 128-row
      contraction chunks; one matmul contracts 256 rows.
  K_stk [128, 8h*1024] fp8: rows 0:64 Kr^T(head cols), 64:128 Ki^T
  Qv1   [128, 8h*512] fp8:  [Qr^T; -Qi^T] per head  (bias folded)
  Qv2   [128, 8h*512] fp8:  [Qi^T;  Qr^T] per head  (bias folded)
  V_all [128, 8kt, 1024] fp8: per k-tile head-paired cols [Vr_h 64|Vi_h 64]
  scores^T psum [128 k, 1024] fp32: cols 0:512 Sr^T, 512:1024 Si^T per kt
  E pair tiles [128, 2, 1024] fp8 = exp(S^T/(1024*8)), dim1 = kt in pair
  P1 psum [128,512] = sum_ktp [Vr|Vi]^T Er (DoubleRow) -> [ErVr; ErVi]
  P2 psum [128,512] -> [EiVr; EiVi]
  r psum [1, 1024]: cols 0:512 sum_k Er, 512: sum_k Ei (ones DR matmuls),
      fanned out to rsb [2, 512] by DMA straight from PSUM.
  OUT_int [128, 4qs, 1024] fp32: final (d,c)-interleaved output tile,
      pre-initialized with residual qc + (1+i)*bv; per-head evac
      accumulates into it (scalar_tensor_tensor adds).
"""

import math
from contextlib import ExitStack

import numpy as np
import ml_dtypes

import concourse.bass as bass
import concourse.tile as tile
from concourse import bacc, mybir
from concourse.bass_utils import run_bass_kernel_spmd
from concourse.masks import make_identity

B, S, D, H = 4, 1024, 512, 8
DH = D // H  # 64
NCORES = 8
TQ = S // 2  # queries per core
EPS = 1e-5
F32 = mybir.dt.float32
F32R = mybir.dt.float32r
BF16 = mybir.dt.bfloat16
F8 = mybir.dt.float8e4
NPF8 = ml_dtypes.float8_e4m3

WS = 32.0  # host-side weight scale (power of 2)
F8MAX = 240.0  # e4m3 (IEEE) max finite

NKT = S // 128  # 8 key tiles
NQS = TQ // 128  # 4 query subtiles
NCH = 8  # contraction chunks (2*D/128)
DR = mybir.MatmulPerfMode.DoubleRow


def build_nc(skip_gb: bool) -> bass.Bass:
    nc = bacc.Bacc(None, target_bir_lowering=False, debug=False)

    xqT_d = nc.declare_dram_parameter("xqT", [128, NCH, TQ], F8, isOutput=False)
    xkT_d = nc.declare_dram_parameter("xkT", [128, NCH, S], F8, isOutput=False)
    xvT_d = nc.declare_dram_parameter("xvT", [128, NCH, S], F8, isOutput=False)
    xqn_d = nc.declare_dram_parameter("xq_nat", [TQ, 2 * D], F32, isOutput=False)
    wqa_d = nc.declare_dram_parameter("wq_a", [128, 8, 1024], F8, isOutput=False)
    wka_d = nc.declare_dram_parameter("wk_a", [128, 4, 2048], F8, isOutput=False)
    wvs_d = nc.declare_dram_parameter("wv_s", [128, 4, 1536], F8, isOutput=False)
    bq1_d = nc.declare_dram_parameter("bq1_stk", [128, H], F32, isOutput=False)
    bv_d = nc.declare_dram_parameter("bv_int", [1, 2 * D], F32, isOutput=False)
    gam_d = nc.declare_dram_parameter("gam_int", [1, 2 * D], F32, isOutput=False)
    bet_d = nc.declare_dram_parameter("bet_int", [1, 2 * D], F32, isOutput=False)
    out_d = nc.declare_dram_parameter("out", [TQ, D, 2], BF16, isOutput=True)

    with tile.TileContext(nc) as tc, ExitStack() as ctx:
        consts = ctx.enter_context(tc.tile_pool(name="consts", bufs=1))
        attn_in = ctx.enter_context(tc.tile_pool(name="attn_in", bufs=1))

        # tiny bias DMAs first (Q evac needs them early; a 2MB transfer
        # queued ahead of them stalls the whole Q stage)
        bq1_stk = consts.tile([128, H], F32)
        nc.sync.dma_start(out=bq1_stk, in_=bq1_d[:])

        # inputs: DMA in compute order (Q stage first so PE starts early);
        # wq_a is head-major so head 0's weights (128KB) land quickly
        wqa = attn_in.tile([128, 8, 1024], F8)
        nc.sync.dma_start(out=wqa[:, 0:1, :], in_=wqa_d[:, 0:1, :])
        xqT = attn_in.tile([128, NCH, TQ], F8)
        nc.sync.dma_start(out=xqT, in_=xqT_d[:])
        for _h in range(1, H):
            nc.sync.dma_start(out=wqa[:, _h : _h + 1, :], in_=wqa_d[:, _h : _h + 1, :])
        wka = attn_in.tile([128, 4, 2048], F8)
        nc.sync.dma_start(out=wka, in_=wka_d[:])
        xkT = attn_in.tile([128, NCH, S], F8)
        nc.sync.dma_start(out=xkT, in_=xkT_d[:])
        wvs = attn_in.tile([128, 4, 1536], F8)
        nc.sync.dma_start(out=wvs, in_=wvs_d[:])
        xvT = attn_in.tile([128, NCH, S], F8)
        nc.sync.dma_start(out=xvT, in_=xvT_d[:])

        ident_f = consts.tile([128, 128], F32)
        make_identity(nc, ident_f)
        ident = consts.tile([128, 128], BF16)
        nc.vector.tensor_copy(out=ident, in_=ident_f)
        ones_f = consts.tile([128, 32], F32)
        nc.vector.memset(ones_f, 1.0)
        ones8 = consts.tile([128, 32], F8)
        nc.vector.tensor_copy(out=ones8, in_=ones_f)
        # DoubleRow ones lhsT [128, 2, 1] (dim1 step 16B for alignment)
        ones_pair = bass.AP(
            tensor=ones8.tensor, offset=ones8.offset,
            ap=[ones8.ap[0], [16, 2], [1, 1]],
        )
        # [0,1] stationary: DR lhsT [128, 2, 2] whose output partition 0
        # receives +0 and partition 1 the ones-reduction (lets r_i share
        # r_r's psum bank one partition over)
        zer_f = consts.tile([128, 32], F32)
        nc.vector.memset(zer_f, 0.0)
        onesB = consts.tile([128, 32], F8)
        nc.vector.tensor_copy(out=onesB, in_=zer_f)
        oneB_col = bass.AP(
            tensor=onesB.tensor, offset=onesB.offset + 1,
            ap=[onesB.ap[0], [16, 2], [1, 1]],
        )
        nc.vector.tensor_copy(out=oneB_col, in_=ones_pair)
        onesB_pair = bass.AP(
            tensor=onesB.tensor, offset=onesB.offset,
            ap=[onesB.ap[0], [16, 2], [1, 2]],
        )
        eps_t = consts.tile([128, 1], F32)
        nc.vector.memset(eps_t, EPS)

        bv_bc = consts.tile([128, 2 * D], F32)
        bcs = [(bv_d, bv_bc)]
        if not skip_gb:
            gam_bc = consts.tile([128, 2 * D], F32)
            bet_bc = consts.tile([128, 2 * D], F32)
            bcs += [(gam_d, gam_bc), (bet_d, bet_bc)]
        for dram, bc in bcs:
            ap0 = dram[:]
            src = bass.AP(tensor=ap0.tensor, offset=0, ap=[[0, 128], [1, 2 * D]])
            nc.gpsimd.dma_start(out=bc, in_=src)

        # attention-phase operand tensors (filled by projection stages)
        K_stk = attn_in.tile([128, H * S], F8)
        V_all = attn_in.tile([128, NKT, 2 * D], F8)
        Qv1 = attn_in.tile([128, H * TQ], F8)
        Qv2 = attn_in.tile([128, H * TQ], F8)
        OUT_int = attn_in.tile([128, NQS, 2 * D], F32)

        # OUT_int <- residual qc (+ deferred (1+i)*bv, added on Pool)
        for qs in range(NQS):
            nc.sync.dma_start(
                out=OUT_int[:, qs, :], in_=xqn_d[qs * 128 : (qs + 1) * 128]
            )
        for qs in range(NQS):
            seg = OUT_int[:, qs, :]
            nc.gpsimd.tensor_add(out=seg, in0=seg, in1=bv_bc)

        def w_pair(w, h, p):
            """DoubleRow lhsT [128, 2, 128] for chunk pair (2p, 2p+1)."""
            pat = 0 if p < 2 else 1
            ds_ = (2 * p) % 4
            return w[:, ds_ : ds_ + 2, pat * 1024 + 128 * h : pat * 1024 + 128 * h + 128]

        def v_rhs(w, p, comp):
            """DoubleRow rhs [128, 2, 512] slot pair for V projection."""
            lo = p < 2
            s = {(0, True): 0, (0, False): 2, (1, True): 1, (1, False): 0}[(comp, lo)]
            ds_ = (2 * p) % 4
            return w[:, ds_ : ds_ + 2, s * 512 : (s + 1) * 512]

        e_pool = ctx.enter_context(tc.tile_pool(name="epool", bufs=7))
        r_pool = ctx.enter_context(tc.tile_pool(name="rsb", bufs=2))
        u_pool = ctx.enter_context(tc.tile_pool(name="usk", bufs=2))
        sc_psum = ctx.enter_context(tc.tile_pool(name="sc_ps", bufs=2, space="PSUM"))
        p1_psum = ctx.enter_context(tc.tile_pool(name="p1_ps", bufs=1, space="PSUM"))
        p2_psum = ctx.enter_context(tc.tile_pool(name="p2_ps", bufs=1, space="PSUM"))
        pj_cm = tc.tile_pool(name="pj_ps", bufs=2, space="PSUM")
        pj_psum = pj_cm.__enter__()

        qt_pool = ctx.enter_context(tc.tile_pool(name="qtmp", bufs=2))

        def q_proj():
            # single projection to [Qr+bqr; Qi+bqi]; Qv1 negates the bottom,
            # Qv2 = swapped halves, moved by two Pool-queue SBUF DMAs
            for h in range(H):
                ps = pj_psum.tile([128, TQ], F32, tag="pj")
                for p in range(4):
                    pat = 0 if p < 2 else 1
                    ds0 = (2 * p) % 4
                    lhsT = bass.AP(
                        tensor=wqa.tensor,
                        offset=wqa.offset + h * 1024 + ds0 * 256 + pat * 128,
                        ap=[wqa.ap[0], [256, 2], [1, 128]],
                    )
                    nc.tensor.matmul(
                        ps, lhsT, xqT[:, 2 * p : 2 * p + 2, :],
                        start=(p == 0), stop=(p == 3),
                        perf_mode=DR,
                    )
                sl1 = Qv1[:, h * TQ : (h + 1) * TQ]
                sl2 = Qv2[:, h * TQ : (h + 1) * TQ]
                qtmp = qt_pool.tile([128, TQ], F8, tag="qtmp", name="qtmp")
                nc.scalar.activation(
                    out=sl1[0:DH, :], in_=ps[0:DH, :],
                    func=mybir.ActivationFunctionType.Identity,
                    bias=bq1_stk[0:DH, h : h + 1],
                )
                nc.scalar.activation(
                    out=qtmp[DH:128, :], in_=ps[DH:128, :],
                    func=mybir.ActivationFunctionType.Identity,
                    bias=bq1_stk[DH:128, h : h + 1],
                )
                nc.vector.tensor_scalar_mul(
                    out=sl1[DH:128, :], in0=qtmp[DH:128, :], scalar1=-1.0
                )
                nc.gpsimd.dma_start(out=sl2[0:DH, :], in_=qtmp[DH:128, :])
                nc.gpsimd.dma_start(out=sl2[DH:128, :], in_=sl1[0:DH, :])

        def k_proj():
            for h in range(H):
                for tch in range(2):
                    ps = pj_psum.tile([128, 512], F32, tag="pj")
                    for p in range(4):
                        nc.tensor.matmul(
                            ps,
                            w_pair(wka, h, p),
                            xkT[:, 2 * p : 2 * p + 2, tch * 512 : (tch + 1) * 512],
                            start=(p == 0), stop=(p == 3),
                            perf_mode=DR,
                        )
                    nc.scalar.copy(
                        out=K_stk[:, h * S + tch * 512 : h * S + (tch + 1) * 512],
                        in_=ps,
                    )

        def v_proj(ts_lo, ts_hi):
            for ts_ in range(ts_lo, ts_hi):
                for comp in range(2):
                    ps = pj_psum.tile([128, 512], F32, tag="pj")
                    for p in range(4):
                        nc.tensor.matmul(
                            ps,
                            xvT[:, 2 * p : 2 * p + 2, ts_ * 128 : ts_ * 128 + 128],
                            v_rhs(wvs, p, comp),
                            start=(p == 0), stop=(p == 3),
                            perf_mode=DR,
                        )
                    # scatter into head-paired layout [Vr_h | Vi_h]
                    dst = bass.AP(
                        tensor=V_all.tensor,
                        offset=V_all.offset + ts_ * 2 * D + comp * DH,
                        ap=[V_all.ap[0], [2 * DH, H], [1, DH]],
                    )
                    nc.vector.tensor_copy(
                        out=dst, in_=ps.rearrange("p (h j) -> p h j", h=H)
                    )

        # ---------------- phase B: attention --------------------------------
        EXPSC = 1.0 / (WS * WS * math.sqrt(DH))
        etiles = {}
        pstate = {}

        def sc_ktp(h, ktp):
            """scores + exp for head h, kt pair ktp."""
            ep = e_pool.tile([128, 2, 1024], F8, tag="e")
            etiles[(h, ktp)] = ep
            for sub in range(2):
                kt = 2 * ktp + sub
                scp = sc_psum.tile([128, 1024], F32, tag="sc")
                klhs = K_stk[:, h * S + kt * 128 : h * S + kt * 128 + 128]
                nc.tensor.matmul(
                    scp[:, 0:TQ], klhs, Qv1[:, h * TQ : (h + 1) * TQ],
                    start=True, stop=True,
                )
                nc.tensor.matmul(
                    scp[:, TQ : 2 * TQ], klhs, Qv2[:, h * TQ : (h + 1) * TQ],
                    start=True, stop=True,
                )
                nc.scalar.activation(
                    out=ep[:, sub, :], in_=scp,
                    func=mybir.ActivationFunctionType.Exp,
                    scale=EXPSC,
                )

        def av_ktp(h, ktp, r_psum):
            """AV + r DoubleRow matmuls for head h, kt pair ktp."""
            if ktp == 0:
                pstate[h] = (
                    p1_psum.tile([128, TQ], F32, tag="p1", name="p1"),
                    p2_psum.tile([128, TQ], F32, tag="p2", name="p2"),
                    r_psum.tile([2, TQ], F32, tag="raux", name="rp"),
                )
            p1, p2, rp = pstate[h]
            ep = etiles.pop((h, ktp))
            vl = V_all[:, 2 * ktp : 2 * ktp + 2, 128 * h : 128 * h + 128]
            erp = ep[:, :, 0:TQ]
            eip = ep[:, :, TQ : 2 * TQ]
            st, sp = (ktp == 0), (ktp == 3)
            nc.tensor.matmul(p1, vl, erp, start=st, stop=sp, perf_mode=DR)
            nc.tensor.matmul(p2, vl, eip, start=st, stop=sp, perf_mode=DR)
            # r_i -> partition 1 (zero col pads partition 0 with +0; its
            # start=True zero runs BEFORE r_r's write), r_r -> partition 0;
            # both share one psum bank
            nc.tensor.matmul(
                rp[0:2, 0:TQ], onesB_pair, eip, start=st, stop=sp,
                perf_mode=DR, skip_group_check=True,
            )
            nc.tensor.matmul(
                rp[0:1, 0:TQ], ones_pair, erp, start=st, stop=sp,
                perf_mode=DR, skip_group_check=True,
            )

        def evac_head(h, r_psum, u_pool, last=False):
            """P1/P2 + r -> OUT_int accumulation for head h."""
            p1, p2, rp = pstate.pop(h)
            # P1 rows [ErVr; ErVi], P2 rows [EiVr; EiVi] (V carries WS)
            usk1 = u_pool.tile([128, TQ], BF16, tag="usk1")
            usk2 = u_pool.tile([128, TQ], BF16, tag="usk2")
            if last:
                # ACT and the sc psum banks are idle once the exp stream
                # drains; keep the tail chain off the busy DVE queue
                nc.scalar.copy(out=usk1, in_=p1)
                nc.scalar.copy(out=usk2, in_=p2)
            else:
                nc.vector.tensor_copy(out=usk1, in_=p1)
                nc.vector.tensor_copy(out=usk2, in_=p2)
            # r rows (x WS so rinv = 1/(WS*r)) -> SBUF, partition-preserving
            rtp = r_pool.tile([2, TQ], BF16, tag="rtp")
            if last:
                nc.scalar.mul(out=rtp, in_=rp, mul=WS)
            else:
                nc.vector.tensor_scalar_mul(out=rtp, in0=rp, scalar1=WS)
            # r transposes first (low cols of utp), then P1/P2 transposes
            utp = r_psum.tile([128, TQ], BF16, tag="utp", name="utp")
            # rinv per qs: [1/(WS r_r), 1/(WS r_i), -1/(WS r_i)]
            rinv = r_pool.tile([128, 3 * NQS], F32, tag="rinv")
            for qs in range(NQS):
                nc.tensor.transpose(
                    utp[:, qs * 2 : qs * 2 + 2],
                    rtp[:, qs * 128 : (qs + 1) * 128],
                    ident[0:2, 0:2],
                )
                nc.vector.reciprocal(
                    out=rinv[:, 3 * qs : 3 * qs + 2],
                    in_=utp[:, qs * 2 : qs * 2 + 2],
                )
                nc.vector.tensor_scalar_mul(
                    out=rinv[:, 3 * qs + 2 : 3 * qs + 3],
                    in0=rinv[:, 3 * qs + 1 : 3 * qs + 2],
                    scalar1=-1.0,
                )
            for qs in range(NQS):
                nc.tensor.transpose(
                    utp[:, qs * 128 : (qs + 1) * 128],
                    usk1[:, qs * 128 : (qs + 1) * 128],
                    ident,
                )
            for qs in range(NQS):
                # pass 1: OUT cols of head h (both comps) += P1t * (1/r_r)
                dst = bass.AP(
                    tensor=OUT_int.tensor,
                    offset=OUT_int.offset + qs * 2 * D + 2 * DH * h,
                    ap=[OUT_int.ap[0], [1, 2], [2, DH]],
                )
                nc.vector.scalar_tensor_tensor(
                    out=dst,
                    in0=utp[:, qs * 128 : (qs + 1) * 128]
                    .rearrange("p (b c) -> p b c", b=2),
                    scalar=rinv[:, 3 * qs : 3 * qs + 1],
                    in1=dst,
                    op0=mybir.AluOpType.mult,
                    op1=mybir.AluOpType.add,
                )
            utp2 = (
                sc_psum.tile([128, TQ], BF16, tag="sc", name="utp2")
                if last else utp
            )
            for qs in range(NQS):
                nc.tensor.transpose(
                    utp2[:, qs * 128 : (qs + 1) * 128],
                    usk2[:, qs * 128 : (qs + 1) * 128],
                    ident,
                )
            for qs in range(NQS):
                # pass 2: r-cols += (EiVi) * (-1/r_i) ; i-cols += (EiVr)/r_i
                for c, sidx in ((0, 2), (1, 1)):
                    dst = bass.AP(
                        tensor=OUT_int.tensor,
                        offset=OUT_int.offset + qs * 2 * D + 2 * DH * h + c,
                        ap=[OUT_int.ap[0], [2, DH]],
                    )
                    src_col = (1 - c) * DH  # r <- EiVi block, i <- EiVr
                    nc.vector.scalar_tensor_tensor(
                        out=dst,
                        in0=utp2[:, qs * 128 + src_col : qs * 128 + src_col + DH],
                        scalar=rinv[:, 3 * qs + sidx : 3 * qs + sidx + 1],
                        in1=dst,
                        op0=mybir.AluOpType.mult,
                        op1=mybir.AluOpType.add,
                    )
                if last:
                    ln_qs(qs)

        ln_pool = ctx.enter_context(tc.tile_pool(name="ln", bufs=8))

        def ln_qs(qs):
            seg = OUT_int[:, qs, :]
            stage = ln_pool.tile([128, 2 * D], BF16, tag="stage", name="stage")
            for c in range(2):
                x = bass.AP(
                    tensor=OUT_int.tensor,
                    offset=OUT_int.offset + qs * 2 * D + c,
                    ap=[OUT_int.ap[0], [2, D]],
                )
                xs = bass.AP(
                    tensor=stage.tensor,
                    offset=stage.offset + c,
                    ap=[stage.ap[0], [2, D]],
                )
                st = ln_pool.tile([128, 6], F32, tag="st", name="st")
                nc.vector.bn_stats(out=st, in_=x)
                mv = ln_pool.tile([128, 2], F32, tag="mv", name="mv")
                nc.vector.bn_aggr(out=mv, in_=st)
                rs = ln_pool.tile([128, 1], F32, tag="rs", name="rs")
                nc.scalar.activation(
                    out=rs, in_=mv[:, 1:2],
                    func=mybir.ActivationFunctionType.Sqrt,
                    bias=eps_t,
                )
                nc.vector.reciprocal(out=rs, in_=rs)
                nc.vector.tensor_scalar(
                    out=x if not skip_gb else xs, in0=x,
                    scalar1=mv[:, 0:1], scalar2=rs,
                    op0=mybir.AluOpType.subtract,
                    op1=mybir.AluOpType.mult,
                )
            if not skip_gb:
                nc.vector.tensor_mul(out=seg, in0=seg, in1=gam_bc)
                nc.vector.tensor_add(out=stage, in0=seg, in1=bet_bc)
            nc.sync.dma_start(
                out=out_d[qs * 128 : (qs + 1) * 128],
                in_=stage.rearrange("p (d c) -> p d c", c=2),
            )

        # ---------------- emission -----------------------------------------
        q_proj()
        k_proj()
        # head 0 scores interleaved with V projection (keeps PE busy while
        # ACT exps head 0)
        for ktp in range(4):
            sc_ktp(0, ktp)
            v_proj(ktp * 2, ktp * 2 + 2)
        # pj pool's PSUM banks are recycled for the r/utp pool
        pj_cm.__exit__(None, None, None)
        r_psum = ctx.enter_context(tc.tile_pool(name="r_ps", bufs=1, space="PSUM"))

        # steady state: head h scores overlap head h-1 AV on the PE stream
        for h in range(1, H):
            for ktp in range(4):
                sc_ktp(h, ktp)
                av_ktp(h - 1, ktp, r_psum)
            evac_head(h - 1, r_psum, u_pool)
        for ktp in range(4):
            av_ktp(H - 1, ktp, r_psum)
        evac_head(H - 1, r_psum, u_pool, last=True)
    nc.compile()
    return nc


def _f8(a):
    return np.clip(a, -F8MAX, F8MAX).astype(NPF8)


def _prep_w(W: np.ndarray):
    """W [D, D, 2] -> (w_a, w_b [128,4,2048], w_s [128,4,1536]) fp8, x WS.

    w_a: pattern1 (Wr^T|Wi^T) head-paired, pattern2 (-Wi^T|Wr^T)
    w_b: pattern3 (Wi^T|Wr^T), pattern4 (Wr^T|-Wi^T)
    w_s: Wr^T | Wi^T | -Wi^T (512-col slots)
    Row dim (d_in) split into 4 sub-blocks -> dim1.
    """
    wr = np.ascontiguousarray(W[:, :, 0].T) * WS  # [d_in, e]
    wi = np.ascontiguousarray(W[:, :, 1].T) * WS

    def paired(a, b):
        out = np.empty((D, 1024), np.float32)
        for h in range(H):
            out[:, 128 * h : 128 * h + DH] = a[:, DH * h : DH * (h + 1)]
            out[:, 128 * h + DH : 128 * h + 128] = b[:, DH * h : DH * (h + 1)]
        return out

    def blk(x):  # [512, n] -> [128, 4, n]
        n = x.shape[1]
        return np.ascontiguousarray(x.reshape(4, 128, n).transpose(1, 0, 2))

    w_a = np.concatenate([paired(wr, wi), paired(-wi, wr)], axis=1)
    w_b = np.concatenate([paired(wi, wr), paired(wr, -wi)], axis=1)
    w_s = np.concatenate([wr, wi, -wi], axis=1)
    return _f8(blk(w_a)), _f8(blk(w_b)), _f8(blk(w_s))


def _xt(x: np.ndarray):
    """x [T, D, 2] f32 -> stacked transposed fp8 [128, 8, T]."""
    T = x.shape[0]
    x8 = _f8(x)
    return np.ascontiguousarray(
        x8.reshape(T, 4, 128, 2).transpose(2, 3, 1, 0).reshape(128, NCH, T)
    )


def _stk(vr, vi):
    """two [H*DH] vectors -> [128, H]: rows 0:64 vr per head, 64:128 vi."""
    out = np.empty((128, H), np.float32)
    for h in range(H):
        out[0:DH, h] = vr[h * DH : (h + 1) * DH]
        out[DH:128, h] = vi[h * DH : (h + 1) * DH]
    return out


def _inter(a, b):
    out = np.empty((1, 2 * D), np.float32)
    out[0, 0::2] = a
    out[0, 1::2] = b
    return out


def host_inputs(inputs: dict) -> tuple[list[dict], bool]:
    q = np.ascontiguousarray(np.asarray(inputs["q"], np.float32))
    k = np.ascontiguousarray(np.asarray(inputs["k"], np.float32))
    v = np.ascontiguousarray(np.asarray(inputs["v"], np.float32))
    Wq, bq = np.asarray(inputs["Wq"], np.float32), np.asarray(inputs["bq"], np.float32)
    Wk = np.asarray(inputs["Wk"], np.float32)
    Wv, bv = np.asarray(inputs["Wv"], np.float32), np.asarray(inputs["bv"], np.float32)
    gr = np.asarray(inputs["gamma_r"], np.float32)
    gi = np.asarray(inputs["gamma_i"], np.float32)
    br = np.asarray(inputs["beta_r"], np.float32)
    bi = np.asarray(inputs["beta_i"], np.float32)
    skip_gb = bool(
        np.all(gr == 1.0) and np.all(gi == 1.0)
        and np.all(br == 0.0) and np.all(bi == 0.0)
    )

    wq_a, _, _ = _prep_w(Wq)
    # head-major repack: [128, 4ds, 2048(pat,128h,c)] -> [128, 8h, 1024(ds,pat,c)]
    wq_a = np.ascontiguousarray(
        wq_a.reshape(128, 4, 2, 8, 128)
        .transpose(0, 3, 1, 2, 4)
        .reshape(128, 8, 1024)
    )
    wk_a, _, _ = _prep_w(Wk)
    _, _, wv_s = _prep_w(Wv)

    shared = {
        "wq_a": wq_a, "wk_a": wk_a, "wv_s": wv_s,
        "bq1_stk": _stk(bq[:, 0], bq[:, 1]) * WS,
        # deferred V-bias: attn rows sum to (1+i), so bv enters as (1+i)*bv
        "bv_int": _inter(bv[:, 0] - bv[:, 1], bv[:, 0] + bv[:, 1]),
        "gam_int": _inter(gr, gi),
        "bet_int": _inter(br, bi),
    }
    in_maps = []
    for c in range(NCORES):
        b_, qh = c // 2, c % 2
        xq = q[b_, qh * TQ : (qh + 1) * TQ]
        in_maps.append(
            {
                "xqT": _xt(xq),
                "xkT": _xt(k[b_]),
                "xvT": _xt(v[b_]),
                "xq_nat": np.ascontiguousarray(xq.reshape(TQ, 2 * D)),
                **shared,
            }
        )
    return in_maps, skip_gb


_NC_CACHE = {}
LAST_RESULT = [None]  # BassKernelResults of the most recent kernel() call


def kernel(**inputs) -> np.ndarray:
    in_maps, skip_gb = host_inputs(inputs)
    if skip_gb not in _NC_CACHE:
        _NC_CACHE[skip_gb] = build_nc(skip_gb)
    nc = _NC_CACHE[skip_gb]
    _NC_CACHE["nc"] = nc  # for test.py tracing
    res = run_bass_kernel_spmd(nc, in_maps, list(range(NCORES)))
    LAST_RESULT[0] = res
    out = np.empty((B, S, D, 2), np.float32)
    for c in range(NCORES):
        b_, qh = c // 2, c % 2
        out[b_, qh * TQ : (qh + 1) * TQ] = np.asarray(
            res.results[c]["out"], dtype=np.float32
        )
    return out
